# revision 2
# baseline (speedup 1.0000x reference)
"""Barrel shifter right 64 (zero-fill) via 16-bit chunk packing, batch 2097152,
8 NeuronCores.

Per 8192-row tile (partition p holds 64 spans of 64 bits):
- Act casts the f32 bit tile to bf16.
- DVE packs: prod = bits * 2^(15 + k%16) (bf16, exact powers of two), then a
  grouped add-reduce gives g = chunk << 15 per 16-bit chunk, written as i32
  (sums of distinct powers 2^15..2^30: exact in f32 accumulation and in the
  i32 convert). rm15 = 15 - (S & 15) comes from a tiny weighted reduce of the
  low four shift bits. Pool converts the two high shift bits (the chunk-shift
  amount q = S >> 4) to i32 cpred masks.
- DVE chunk math: two in-place predicated copies gather d[j] = c[j-q];
  W = (g[j-1] >> 16) | g[j] = (d[j]<<16 | d[j-1]) >> 1; e = W >> (15-r)
  leaves the shifted output chunk in bits 0..15; X = (e & 0xFFFF) |
  ((e>>1)<<16) so a single shift by 2u extracts bit 2u at bit0 and bit 2u+1
  at bit16; V = X >> iota(0,2,..,14); V16 &= 1.
- Act reads V (i16) directly and writes the f32 output tile.
"""

import sys

if "/opt/trn_rl_repo" not in sys.path:
    sys.path.insert(0, "/opt/trn_rl_repo")

import numpy as np

B_TOTAL = 2097152
NBITS = 64
NCTRL = 6
NCORES = 8
R_FULL = B_TOTAL // NCORES  # 262144 rows per core

P = 128
SPANS = 64                  # rows per partition per tile
TILE_ROWS = P * SPANS       # 8192
FD = SPANS * NBITS          # 4096
SFD = SPANS * NCTRL         # 384
NCH = 4                     # 16-bit chunks per row
CPITCH = NCH + 2            # guard(2) + chunks(4) per span in the chunk buffer
SLOTS = 3

_built = {}


def build(rows):
    import concourse.bass as bass
    from concourse import mybir

    f32 = mybir.dt.float32
    bf16 = mybir.dt.bfloat16
    i16 = mybir.dt.int16
    i32 = mybir.dt.int32
    Alu = mybir.AluOpType
    ActF = mybir.ActivationFunctionType

    nt = rows // TILE_ROWS
    assert rows % TILE_ROWS == 0

    nc = bass.Bass(detect_race_conditions=False)
    data = nc.declare_dram_parameter("data", [rows, NBITS], f32, isOutput=False)
    shift = nc.declare_dram_parameter("shift", [rows, NCTRL], f32, isOutput=False)
    out = nc.declare_dram_parameter("out", [rows, NBITS], f32, isOutput=True)

    data_r = data.rearrange("(n p t) k -> n p (t k)", p=P, t=SPANS)
    shift_r = shift.rearrange("(n p t) k -> n p (t k)", p=P, t=SPANS)
    out_r = out.rearrange("(n p t) k -> n p (t k)", p=P, t=SPANS)

    def sb(name, shape, dt):
        return nc.alloc_sbuf_tensor(name, shape, dt)

    dtile = [sb(f"dtile{j}", [P, FD], f32) for j in range(SLOTS)]
    stile = [sb(f"stile{j}", [P, SFD], f32) for j in range(SLOTS)]
    dbf = [sb(f"dbf{j}", [P, FD], bf16) for j in range(SLOTS)]
    prod = sb("prod", [P, FD], bf16)
    W30 = sb("W30", [P, NBITS], bf16)
    W4 = sb("W4", [P, NCH], f32)
    w4p = sb("w4p", [P, SPANS * NCH], f32)
    rf = sb("rf", [P, SPANS], f32)
    rm15 = sb("rm15", [P, SPANS], i32)
    mA32 = [sb(f"mA32_{j}", [P, SPANS], i32) for j in range(SLOTS)]
    mB32 = [sb(f"mB32_{j}", [P, SPANS], i32) for j in range(SLOTS)]
    C32 = [sb(f"C32_{j}", [P, SPANS * CPITCH], i32) for j in range(SLOTS)]
    Wb = sb("Wb", [P, SPANS * NCH], i32)
    A2 = sb("A2", [P, SPANS * NCH], i32)
    Xb = sb("Xb", [P, SPANS * NCH], i32)
    Vb = [sb(f"Vb_{j}", [P, FD], i16) for j in range(SLOTS)]
    IOTAE = sb("IOTAE", [P, 8], i32)
    cK16 = sb("cK16", [P, 1], i32)
    cKF = sb("cKF", [P, 1], i32)
    otile = [sb(f"otile{j}", [P, FD], f32) for j in range(SLOTS)]

    def chunks(s, off=0):
        # [P, SPANS, NCH] view at chunk positions [off, NCH+off) (chunk 0 at
        # buffer position 2; zero guards at 0..1)
        return C32[s].ap().rearrange("p (t c) -> p t c", c=CPITCH)[
            :, :, 2 + off:2 + NCH + off
        ]

    with (
        nc.Block() as block,
        nc.semaphore("s_din0") as s_din0,
        nc.semaphore("s_din1") as s_din1,
        nc.semaphore("s_din2") as s_din2,
        nc.semaphore("s_dout0") as s_dout0,
        nc.semaphore("s_dout1") as s_dout1,
        nc.semaphore("s_dout2") as s_dout2,
        nc.semaphore("s_actin") as s_actin,
        nc.semaphore("s_pool") as s_pool,
        nc.semaphore("s_dve") as s_dve,
        nc.semaphore("s_act") as s_act,
    ):
        s_din = [s_din0, s_din1, s_din2]
        s_dout = [s_dout0, s_dout1, s_dout2]

        @block.sync
        def _(sp):
            for n in range(nt):
                c = n % SLOTS
                if n >= SLOTS:
                    sp.wait_ge(s_actin, n - SLOTS + 1)  # dtile[c] free
                    sp.wait_ge(s_dve, n - SLOTS + 1)    # stile[c] free (w4p)
                    sp.wait_ge(s_pool, n - SLOTS + 1)   # stile[c] free (masks)
                sp.dma_start(
                    out=dtile[c].ap(), in_=data_r[n]
                ).then_inc(s_din[c], 16)
                sp.dma_start(
                    out=stile[c].ap(), in_=shift_r[n]
                ).then_inc(s_din[c], 16)
                if n >= 2:
                    m = n - 2
                    sp.wait_ge(s_act, m + 1)
                    sp.dma_start(
                        out=out_r[m], in_=otile[m % SLOTS].ap()
                    ).then_inc(s_dout[m % SLOTS], 16)
            for m in (nt - 2, nt - 1):
                sp.wait_ge(s_act, m + 1)
                sp.dma_start(
                    out=out_r[m], in_=otile[m % SLOTS].ap()
                ).then_inc(s_dout[m % SLOTS], 16)
            for j in range(SLOTS):
                ndone = len([n for n in range(nt) if n % SLOTS == j])
                sp.wait_ge(s_dout[j], 16 * ndone)

        @block.scalar
        def _(a):
            def outcast(m):
                a.wait_ge(s_dve, m + 1)
                if m >= SLOTS:
                    a.wait_ge(s_dout[m % SLOTS], 16 * (m // SLOTS))
                a.activation(
                    otile[m % SLOTS].ap(), Vb[m % SLOTS].ap(), ActF.Copy
                ).then_inc(s_act, 1)

            for n in range(nt):
                c = n % SLOTS
                # full 32(k+1): the s_din counter is shared by the data and
                # shift DMAs and they can complete in either order, so a
                # partial 32k+16 target can be met by the small shift DMA
                # while dtile is still streaming
                a.wait_ge(s_din[c], 32 * (n // SLOTS + 1))
                if n >= SLOTS:
                    a.wait_ge(s_dve, n - SLOTS + 1)  # dbf[c] free
                a.activation(
                    dbf[c].ap(), dtile[c].ap(), ActF.Copy
                ).then_inc(s_actin, 1)
                if n >= 1:
                    outcast(n - 1)
            outcast(nt - 1)

        @block.gpsimd
        def _(g):
            for n in range(nt):
                c = n % SLOTS
                g.wait_ge(s_din[c], 32 * (n // SLOTS + 1))
                if n >= SLOTS:
                    g.wait_ge(s_dve, n - SLOTS + 1)  # mask buffers free
                # q-bit masks: raw shift lanes 1 (16s) and 0 (32s) -> i32
                st3 = stile[c].ap().rearrange("p (t j) -> p t j", j=NCTRL)
                g.tensor_copy(mA32[c].ap().unsqueeze(2), st3[:, :, 1:2])
                g.tensor_copy(
                    mB32[c].ap().unsqueeze(2), st3[:, :, 0:1]
                ).then_inc(s_pool, 1)

        @block.vector
        def _(v):
            # constants
            W30v = W30.ap().rearrange("p (g j) -> p g j", j=16)
            for j in range(16):
                v.memset(W30v[:, :, j:j + 1], float(1 << (15 + j)))
            for j in range(NCH):
                v.memset(W4.ap()[:, j:j + 1], float(8 >> j))
            for j in range(SLOTS):
                v.memset(C32[j].ap(), 0)
            for u in range(8):
                v.memset(IOTAE.ap()[:, u:u + 1], 2 * u)
            v.memset(cK16.ap(), 16)
            v.memset(cKF.ap(), 0xFFFF)
            for n in range(nt):
                c = n % SLOTS
                v.wait_ge(s_actin, n + 1)
                v.wait_ge(s_din[c], 32 * (n // SLOTS + 1))
                if n >= SLOTS:
                    v.wait_ge(s_act, n - SLOTS + 1)  # Vb[c] free
                # every short-op producer below is followed by a long op (or
                # >=1us of unrelated work) before its consumer reads it — DVE
                # writes from sub-~300ns ops don't drain before the next
                # instruction issues (see baseline kernel notes)
                st3 = stile[c].ap().rearrange("p (t j) -> p t j", j=NCTRL)
                W4b = W4.ap().unsqueeze(1).broadcast_to([P, SPANS, NCH])
                v.tensor_tensor(
                    w4p.ap().rearrange("p (t c) -> p t c", c=NCH),
                    st3[:, :, 2:6], W4b, Alu.mult,
                )
                # pack: bits * 2^(15 + k%16), grouped add-reduce -> g = c<<15
                # (long: spaces w4p from rf)
                d3 = dbf[c].ap().rearrange("p (t k) -> p t k", k=NBITS)
                W30b = W30.ap().unsqueeze(1).broadcast_to([P, SPANS, NBITS])
                v.tensor_tensor(
                    prod.ap().rearrange("p (t k) -> p t k", k=NBITS),
                    d3, W30b, Alu.mult,
                )
                v.tensor_reduce(
                    rf.ap(),
                    w4p.ap().rearrange("p (t c) -> p t c", c=NCH),
                    mybir.AxisListType.X, Alu.add,
                )
                # (long: spaces rf from rm15)
                with nc.allow_low_precision(
                    reason="sums of distinct powers of two; exact in f32 and i32"
                ):
                    v.tensor_reduce(
                        chunks(c),
                        prod.ap().rearrange("p (t c k) -> p t c k", c=NCH, k=16),
                        mybir.AxisListType.X, Alu.add,
                    )
                # 15 - r, f32 -> i32
                v.tensor_scalar(rm15.ap(), rf.ap(), -1.0, 15.0,
                                Alu.mult, Alu.add)
                # q-gather: d[j] = c[j-q]
                v.wait_ge(s_pool, n + 1)
                mAb = mA32[c].ap().unsqueeze(2).broadcast_to([P, SPANS, NCH])
                mBb = mB32[c].ap().unsqueeze(2).broadcast_to([P, SPANS, NCH])
                v.copy_predicated(chunks(c), mAb, chunks(c, off=-1))
                # spacer: flush cpredA's writes before cpredB reads them
                v.memset(w4p.ap(), 0)
                v.copy_predicated(chunks(c), mBb, chunks(c, off=-2))
                # W = (g[j-1] >> 16) | g[j]  (= (d[j]<<16 | d[j-1]) >> 1)
                W3 = Wb.ap().rearrange("p (t c) -> p t c", c=NCH)
                v.scalar_tensor_tensor(
                    W3, chunks(c, off=-1), cK16.ap(), chunks(c),
                    Alu.logical_shift_right, Alu.bitwise_or,
                )
                # e = W >> (15 - r), in place (bits 0..15 = shifted chunk)
                rmb = rm15.ap().unsqueeze(2).broadcast_to([P, SPANS, NCH])
                v.tensor_tensor(W3, W3, rmb, Alu.logical_shift_right)
                # X = (e & 0xFFFF) | ((e >> 1) << 16)
                A3 = A2.ap().rearrange("p (t c) -> p t c", c=NCH)
                X3 = Xb.ap().rearrange("p (t c) -> p t c", c=NCH)
                v.tensor_scalar(A3, W3, 1, 16,
                                Alu.logical_shift_right, Alu.logical_shift_left)
                v.scalar_tensor_tensor(
                    X3, W3, cKF.ap(), A3, Alu.bitwise_and, Alu.bitwise_or,
                )
                # V = X >> (0,2,...,14): bit 2u at bit0, bit 2u+1 at bit16
                v.tensor_tensor(
                    Vb[c].ap().bitcast(i32).rearrange("p (t c u) -> p t c u",
                                                      c=NCH, u=8),
                    Xb.ap().rearrange("p (t c) -> p t c", c=NCH)
                    .unsqueeze(3).broadcast_to([P, SPANS, NCH, 8]),
                    IOTAE.ap().unsqueeze(1).unsqueeze(2)
                    .broadcast_to([P, SPANS, NCH, 8]),
                    Alu.logical_shift_right,
                )
                v.tensor_scalar(
                    Vb[c].ap(), Vb[c].ap(), 1, None, Alu.bitwise_and
                ).then_inc(s_dve, 1)

    return nc


def _get(rows):
    if rows not in _built:
        _built[rows] = build(rows)
    return _built[rows]


def run_cores(data, shift, rows, trace=False):
    from concourse.bass_utils import run_bass_kernel_spmd

    nc = _get(rows)
    ncores = data.shape[0] // rows
    in_maps = [
        {
            "data": np.ascontiguousarray(data[i * rows:(i + 1) * rows]),
            "shift": np.ascontiguousarray(shift[i * rows:(i + 1) * rows]),
        }
        for i in range(ncores)
    ]
    res = run_bass_kernel_spmd(nc, in_maps, list(range(ncores)), trace=trace)
    full = np.concatenate([res.results[i]["out"] for i in range(ncores)], axis=0)
    return full, res


def kernel(data, shift):
    data = np.ascontiguousarray(np.asarray(data), dtype=np.float32)
    shift = np.ascontiguousarray(np.asarray(shift), dtype=np.float32)
    full, _ = run_cores(data, shift, R_FULL)
    return full.astype(np.float32, copy=False)
